# revision 23
# baseline (speedup 1.0000x reference)
"""Trainium2 Bass kernel for nn_ODE_71743133713072.

Semantics (unrolled from the reference lax.scan):
  out[:, 0]   = lat[:, 0]
  out[:, 2]   = lat[:, 1]                                (dt=0 scan quirk)
  out[:, t+1] = lat[:, t] + h * f(lat[:, t])   for t = 0, 2..99
  y = out[:, 100];  out[:, k+1] = y = y + h * f(y)  for k = 100..118
where f is the D->U->U->D tanh MLP and h = ts[1]-ts[0] (linspace; per-step
fp32 diffs differ from h by <=1 ulp, far below the fp8 matmul noise floor).

Everything on-device runs FEATURE-MAJOR ([d, batch*time]); the host
pre-transposes the inputs and post-transposes the outputs, so the PE does
zero transposes and zero bias matmuls:
  - xT8:   fp8(lat^T)  -> L1 moving operand, DoubleRow K=256.
  - latTB: lat^T + h*b3 (f32) -> the Euler-add operand (b3 host-folded).
  - L1/L2/L3 keep the (fp8, x8-scaled) weights stationary; activations are
    always the moving operand, so no role swap and N=512 per matmul.
  - Euler update is one fused stt: oT = mm3 * (h/8) + latTB.
Frames 0 and 2 are copied from lat on the host.  The 19-step prediction
chain stays feature-major (state = oT[g=24] tail slice); its MLP biases
enter via DVE-seeded PSUM (matmul start=False accumulates on top), so each
act/stt is a single merged instruction and the serial path per step is
stt_fp8 -> 2 MM -> act -> 2 MM -> act -> 2 MM -> stt_fp8.

The chain is latency-bound while the stream is throughput-bound; in-order
engine queues head-of-line block if either is emitted in large runs.  The
emission therefore interleaves ONE chain step with ONE stream group at
matching sub-stage granularity (chain MMs just before group MMs, chain act
before group acts, ...), keeping parked chain ops within each engine's
wait-queue depth so ready stream work flows around them.
"""

import os
import sys
from contextlib import ExitStack

import numpy as np

for _p in ("/opt/trn_rl_repo", "/root/.axon_site/_ro/trn_rl_repo"):
    if os.path.isdir(_p) and _p not in sys.path:
        sys.path.append(_p)

import ml_dtypes  # noqa: E402

B, T_OBS, KPRED, D = 1024, 100, 20, 256
T = T_OBS + KPRED          # 120
NCORES = 8
PB = B // NCORES           # 128 rows per core
P = 128
G = 4                      # time steps per compute group
NG = T_OBS // G            # 25 groups
NCH = KPRED - 1            # 19 chain steps


def _emit(ctx, tc, latTB, xT8, w8d, bpkd, bseedd, outT, outR, h):
    import concourse.mybir as mybir

    nc = tc.nc
    F32 = mybir.dt.float32
    BF16 = mybir.dt.bfloat16
    FP8 = mybir.dt.float8e4
    AF = mybir.ActivationFunctionType
    ALU = mybir.AluOpType
    DR = mybir.MatmulPerfMode.DoubleRow

    const = ctx.enter_context(tc.tile_pool(name="const", bufs=1))
    # preload the tanh act table while the first DMAs are in flight
    with tc.high_priority():
        scr = const.tile([P, 2], F32, tag="scr")
        nc.vector.memset(scr[:, 0:1], 0.0)
        nc.scalar.activation(scr[:, 1:2], scr[:, 0:1],
                             mybir.ActivationFunctionType.Tanh)
    # fp8 weights (x8-scaled), stationary layout [K_lo, ktile, M]; w1 first so
    # the chain-bootstrap mini group can start before w2/w3 arrive
    w8sb = const.tile([P, 3, 2, D], FP8, tag="w8")
    nc.sync.dma_start(w8sb[:, 0, :, :],
                      w8d[0].rearrange("(k p) m -> p k m", k=2))
    # PSUM bias seeds for the chain: 8*b1 / 8*b2 / 8*b3, bcast along batch
    bseed = const.tile([P, 3, 2, P], BF16, tag="bseed")
    nc.scalar.dma_start(bseed[:], bseedd[:])
    for wi in (1, 2):
        nc.sync.dma_start(w8sb[:, wi, :, :],
                          w8d[wi].rearrange("(k p) m -> p k m", k=2))
    bsb = const.tile([P, 4], F32, tag="bias")
    nc.scalar.dma_start(bsb[:], bpkd[:])

    b1ap = [bsb[:, 0:1], bsb[:, 1:2]]
    b2ap = [bsb[:, 2:3], bsb[:, 3:4]]

    latp = ctx.enter_context(tc.tile_pool(name="lat", bufs=5))
    x8p = ctx.enter_context(tc.tile_pool(name="x8", bufs=5))
    h1p = ctx.enter_context(tc.tile_pool(name="h1", bufs=4))
    h2p = ctx.enter_context(tc.tile_pool(name="h2", bufs=4))
    oTp = ctx.enter_context(tc.tile_pool(name="oT", bufs=4))
    ringp = ctx.enter_context(tc.tile_pool(name="ring", bufs=1))
    y8pool = ctx.enter_context(tc.tile_pool(name="y8", bufs=4))
    c1sp = ctx.enter_context(tc.tile_pool(name="c1s", bufs=4))
    c2sp = ctx.enter_context(tc.tile_pool(name="c2s", bufs=4))
    minip = ctx.enter_context(tc.tile_pool(name="mini", bufs=1))

    # one PSUM bank per tile: fine-grained recycling so a stream matmul never
    # parks long at the PE queue head waiting for an act to free a bank
    mmps = ctx.enter_context(tc.tile_pool(name="mmps", bufs=6, space="PSUM"))
    chps = ctx.enter_context(tc.tile_pool(name="chps", bufs=2, space="PSUM"))

    h8 = float(h / 8.0)

    # ---- stream group stages -------------------------------------------
    def g_load(g):
        t0 = g * G
        xt = latp.tile([P, 2, G, P], F32, tag="lat")
        nc.sync.dma_start(xt[:], latTB[:, :, t0:t0 + G, :])
        x8 = x8p.tile([P, 2, G, P], FP8, tag="x8")
        nc.gpsimd.dma_start(x8[:], xT8[:, :, t0:t0 + G, :])
        return dict(xt=xt, x8=x8)

    def g_mm(s, wi, key, src):
        # N=256 halves: a chain matmul arriving at the PE queue waits out at
        # most ~256 rows of in-flight stream work instead of ~512
        mm = []
        NH = G * P // 4
        for mc in range(2):
            t = mmps.tile([P, G * P], F32, tag="mm", name=f"mm{wi}_{mc}")
            for nh in range(4):
                nc.tensor.matmul(t[:, nh * NH:(nh + 1) * NH],
                                 w8sb[:, wi, :, mc * P:(mc + 1) * P],
                                 src[:, :, nh * NH:(nh + 1) * NH],
                                 start=True, stop=True, perf_mode=DR)
            mm.append(t)
        s[key] = mm

    def g_act(s, mmkey, key, pool, bap):
        ht = pool.tile([P, 2, G * P], FP8, tag=key)
        for mc in range(2):
            nc.scalar.activation(ht[:, mc, :], s[mmkey][mc][:], AF.Tanh,
                                 bias=bap[mc], scale=0.125)
        s[key] = ht

    def g_store(s, g):
        t0 = g * G
        oT = oTp.tile([P, 2, G * P], F32, tag="oT")
        xtf = s["xt"].rearrange("p a t b -> p a (t b)")
        NH = G * P // 2
        for dc in range(2):
            for nh in range(2):
                sl = slice(nh * NH, (nh + 1) * NH)
                nc.vector.scalar_tensor_tensor(
                    oT[:, dc, sl], s["mm3"][dc][:, sl], h8, xtf[:, dc, sl],
                    ALU.mult, ALU.add)
        nc.sync.dma_start(outT[:, :, t0:t0 + G, :],
                          oT.rearrange("p a (t b) -> p a t b", t=G))
        s["oT"] = oT

    def g_all(s, g):
        g_mm(s, 0, "mm1", s["x8"].rearrange("p k t b -> p k (t b)"))
        g_act(s, "mm1", "h1", h1p, b1ap)
        g_mm(s, 1, "mm2", s["h1"])
        g_act(s, "mm2", "h2", h2p, b2ap)
        g_mm(s, 2, "mm3", s["h2"])
        g_store(s, g)

    # ---- chain stages ---------------------------------------------------
    def c_mm(wi, src, name):
        t = chps.tile([P, 2, P], F32, tag="ch", name=name)
        nc.vector.tensor_copy(t[:], bseed[:, wi, :, :])
        for mc in range(2):
            nc.tensor.matmul(t[:, mc, :],
                             w8sb[:, wi, :, mc * P:(mc + 1) * P],
                             src[:], start=False, stop=True, perf_mode=DR,
                             skip_group_check=True)
        return t

    def c_act(cin, pool, tag):
        t = pool.tile([P, 2, P], FP8, tag=tag)
        nc.scalar.activation(t[:], cin[:], AF.Tanh, scale=0.125)
        return t

    # ---- schedule -------------------------------------------------------
    # mini pre-group: compute y0 = out[:,100] from t=99 alone so the chain
    # starts ~6us in, instead of waiting for all of group 24
    with tc.high_priority():
        xt99 = minip.tile([P, 2, 1, P], F32, tag="xt99")
        nc.sync.dma_start(xt99[:], latTB[:, :, T_OBS - 1:T_OBS, :])
        x899 = minip.tile([P, 2, 1, P], FP8, tag="x899")
        nc.gpsimd.dma_start(x899[:], xT8[:, :, T_OBS - 1:T_OBS, :])
        m1 = c_mm(0, x899.rearrange("p a t b -> p a (t b)"), "m1")
        m1s = c_act(m1, c1sp, "c1s")
        m2 = c_mm(1, m1s, "m2")
        m2s = c_act(m2, c2sp, "c2s")
        m3 = chps.tile([P, 2, P], F32, tag="ch", name="m3")
        for mc in range(2):
            nc.tensor.matmul(m3[:, mc, :], w8sb[:, 2, :, mc * P:(mc + 1) * P],
                             m2s[:], start=True, stop=True, perf_mode=DR)
        y0 = minip.tile([P, 2, P], F32, tag="y0")
        nc.vector.scalar_tensor_tensor(y0[:], m3[:], h8,
                                       xt99.rearrange("p a t b -> p a (t b)"),
                                       ALU.mult, ALU.add)

        ring = ringp.tile([P, NCH, 2, P], F32, tag="ring")
        ysrc = y0[:, :, :]
        y8 = y8pool.tile([P, 2, P], FP8, tag="y8")
        nc.vector.tensor_copy(y8[:], ysrc)

    order = [NG - 1] + list(range(NG - 1))
    counts = [1] * 13 + [2] * 6          # 25 groups over 19 chain slots
    assert sum(counts) == len(order)
    states = {}
    nloaded = 0

    def prefetch(upto):
        nonlocal nloaded
        while nloaded < min(upto, len(order)):
            states[order[nloaded]] = g_load(order[nloaded])
            nloaded += 1

    prefetch(2)
    done = 0
    for k in range(NCH):
        gs = [states[order[done + i]] for i in range(counts[k])]
        done += counts[k]
        prefetch(done + counts[min(k + 1, NCH - 1)] + 1)
        with tc.high_priority():
            c1 = c_mm(0, y8, "c1")
        for s in gs:
            g_mm(s, 0, "mm1", s["x8"].rearrange("p k t b -> p k (t b)"))
        with tc.high_priority():
            c1s = c_act(c1, c1sp, "c1s")
        for s in gs:
            g_act(s, "mm1", "h1", h1p, b1ap)
        with tc.high_priority():
            c2 = c_mm(1, c1s, "c2")
        for s in gs:
            g_mm(s, 1, "mm2", s["h1"])
        with tc.high_priority():
            c2s = c_act(c2, c2sp, "c2s")
        for s in gs:
            g_act(s, "mm2", "h2", h2p, b2ap)
        with tc.high_priority():
            c3 = c_mm(2, c2s, "c3")
        for s in gs:
            g_mm(s, 2, "mm3", s["h2"])
        ynew = ring[:, k, :, :]
        with tc.high_priority():
            if k < NCH - 1:
                y8n = y8pool.tile([P, 2, P], FP8, tag="y8")
                nc.vector.scalar_tensor_tensor(y8n[:], c3[:], h8, ysrc,
                                               ALU.mult, ALU.add)
                y8 = y8n
            nc.vector.scalar_tensor_tensor(ynew, c3[:], h8, ysrc,
                                           ALU.mult, ALU.add)
        for i, s in enumerate(gs):
            g_store(s, order[done - counts[k] + i])
        ysrc = ynew
        if k == 12:
            nc.gpsimd.dma_start(outR[:, 0:12, :, :], ring[:, 0:12, :, :])
        elif k == 16:
            nc.gpsimd.dma_start(outR[:, 12:16, :, :], ring[:, 12:16, :, :])

    nc.gpsimd.dma_start(outR[:, 16:, :, :], ring[:, 16:, :, :])


def _build(h):
    import concourse.mybir as mybir
    import concourse.tile as tile
    from concourse import bacc

    F32 = mybir.dt.float32
    BF16 = mybir.dt.bfloat16
    FP8 = mybir.dt.float8e4

    nc = bacc.Bacc("TRN2", target_bir_lowering=False, debug=False,
                   num_devices=NCORES)
    latTB = nc.dram_tensor("latTB", [P, 2, T_OBS, PB], F32,
                           kind="ExternalInput").ap()
    xT8 = nc.dram_tensor("xT8", [P, 2, T_OBS, PB], FP8,
                         kind="ExternalInput").ap()
    w8d = nc.dram_tensor("w8", [3, D, D], FP8, kind="ExternalInput").ap()
    bpkd = nc.dram_tensor("bpack", [P, 4], F32, kind="ExternalInput").ap()
    bseedd = nc.dram_tensor("bseed", [P, 3, 2, PB], BF16,
                            kind="ExternalInput").ap()
    outT = nc.dram_tensor("outT", [P, 2, T_OBS, PB], F32,
                          kind="ExternalOutput").ap()
    outR = nc.dram_tensor("outR", [P, NCH, 2, PB], F32,
                          kind="ExternalOutput").ap()

    with tile.TileContext(nc) as tc, ExitStack() as ctx:
        _emit(ctx, tc, latTB, xT8, w8d, bpkd, bseedd, outT, outR, h)
    nc.compile()
    return nc


def _host_inputs(inputs):
    ts = np.asarray(inputs["time_steps"], np.float32)
    h = float(np.float32(ts[1]) - np.float32(ts[0]))

    f8 = ml_dtypes.float8_e4m3
    W1 = np.asarray(inputs["W1"], np.float32)
    W2 = np.asarray(inputs["W2"], np.float32)
    W3 = np.asarray(inputs["W3"], np.float32)
    b1 = np.asarray(inputs["b1"], np.float32)
    b2 = np.asarray(inputs["b2"], np.float32)
    b3 = np.asarray(inputs["b3"], np.float32)
    w8 = np.stack([8.0 * W1, 8.0 * W2, 8.0 * W3]).astype(f8)
    b3h = (b3 * np.float32(h)).astype(np.float32)
    bpack = np.stack([b1[:P], b1[P:], b2[:P], b2[P:]],
                     axis=1).astype(np.float32)
    # [P, 3, 2, PB]: 8*b{1,2,3}[mc*128+p] broadcast along batch
    bs = np.stack([8.0 * b1, 8.0 * b2, 8.0 * b3])        # [3, 256]
    bseed = np.ascontiguousarray(
        np.broadcast_to(bs.reshape(3, 2, P, 1).transpose(2, 0, 1, 3),
                        (P, 3, 2, PB))).astype(ml_dtypes.bfloat16)
    shared = dict(w8=w8, bpack=bpack, bseed=bseed)
    return h, shared, b3h


def _percore_inputs(lat_full, b3h):
    # lat_full [B, T_OBS, D] -> per-core latTB/xT8 [P, 2, T_OBS, PB]
    f8 = ml_dtypes.float8_e4m3
    x = lat_full.reshape(NCORES, PB, T_OBS, 2, P)   # [c, b, t, dc, p]
    xt = x.transpose(0, 4, 3, 2, 1)                 # [c, p, dc, t, b]
    b3r = b3h.reshape(2, P).transpose(1, 0)         # [p, dc]
    latTBs = (xt + b3r[None, :, :, None, None]).astype(np.float32)
    xT8s = np.ascontiguousarray(xt).astype(f8)
    return latTBs, xT8s


def _assemble(lat_full, results):
    out = np.empty((B, T, D), np.float32)
    for c in range(NCORES):
        sl = slice(c * PB, (c + 1) * PB)
        oT = results[c]["outT"]    # [P, 2, T_OBS, PB]
        oR = results[c]["outR"]    # [P, NCH, 2, PB]
        out[sl, 1:T_OBS + 1, :] = oT.transpose(3, 2, 1, 0).reshape(
            PB, T_OBS, D)
        out[sl, T_OBS + 1:, :] = oR.transpose(3, 1, 2, 0).reshape(PB, NCH, D)
    out[:, 0, :] = lat_full[:, 0, :]
    out[:, 2, :] = lat_full[:, 1, :]
    return out


_CACHE = {}


def make_in_maps(inputs):
    lat_full = np.ascontiguousarray(np.asarray(inputs["latents"], np.float32))
    h, shared, b3h = _host_inputs(inputs)
    if h not in _CACHE:
        _CACHE[h] = _build(h)
    nc = _CACHE[h]
    latTBs, xT8s = _percore_inputs(lat_full, b3h)
    in_maps = []
    for c in range(NCORES):
        m = dict(shared)
        m["latTB"] = np.ascontiguousarray(latTBs[c])
        m["xT8"] = xT8s[c]
        in_maps.append(m)
    return nc, in_maps, lat_full


def kernel(**inputs):
    from concourse.bass_utils import run_bass_kernel_spmd

    nc, in_maps, lat_full = make_in_maps(inputs)
    res = run_bass_kernel_spmd(nc, in_maps, list(range(NCORES)))
    return _assemble(lat_full, [res.results[c] for c in range(NCORES)])


# revision 25
# speedup vs baseline: 1.0956x; 1.0956x over previous
"""Trainium2 Bass kernel for nn_ODE_71743133713072.

Semantics (unrolled from the reference lax.scan):
  out[:, 0]   = lat[:, 0]
  out[:, 2]   = lat[:, 1]                                (dt=0 scan quirk)
  out[:, t+1] = lat[:, t] + h * f(lat[:, t])   for t = 0, 2..99
  y = out[:, 100];  out[:, k+1] = y = y + h * f(y)  for k = 100..118
where f is the D->U->U->D tanh MLP and h = ts[1]-ts[0] (linspace; per-step
fp32 diffs differ from h by <=1 ulp, far below the fp8 matmul noise floor).

Everything on-device runs FEATURE-MAJOR ([d, batch*time]); the host
pre-transposes the inputs and post-transposes the outputs, so the PE does
zero transposes and zero bias matmuls:
  - xT8:   fp8(lat^T)  -> L1 moving operand, DoubleRow K=256.
  - latTB: lat^T + h*b3 (f32) -> the Euler-add operand (b3 host-folded).
  - L1/L2/L3 keep the (fp8, x8-scaled) weights stationary; activations are
    always the moving operand, so no role swap and N=512 per matmul.
  - Euler update is one fused stt: oT = mm3 * (h/8) + latTB.
Frames 0 and 2 are copied from lat on the host.  The 19-step prediction
chain stays feature-major (state = oT[g=24] tail slice); its MLP biases
enter via DVE-seeded PSUM (matmul start=False accumulates on top), so each
act/stt is a single merged instruction and the serial path per step is
stt_fp8 -> 2 MM -> act -> 2 MM -> act -> 2 MM -> stt_fp8.

The chain is latency-bound while the stream is throughput-bound; in-order
engine queues head-of-line block if either is emitted in large runs.  The
emission therefore interleaves ONE chain step with ONE stream group at
matching sub-stage granularity (chain MMs just before group MMs, chain act
before group acts, ...), keeping parked chain ops within each engine's
wait-queue depth so ready stream work flows around them.
"""

import os
import sys
from contextlib import ExitStack

import numpy as np

for _p in ("/opt/trn_rl_repo", "/root/.axon_site/_ro/trn_rl_repo"):
    if os.path.isdir(_p) and _p not in sys.path:
        sys.path.append(_p)

import ml_dtypes  # noqa: E402

B, T_OBS, KPRED, D = 1024, 100, 20, 256
T = T_OBS + KPRED          # 120
NCORES = 8
PB = B // NCORES           # 128 rows per core
P = 128
G = 4                      # time steps per compute group
NG = T_OBS // G            # 25 groups
NCH = KPRED - 1            # 19 chain steps


def _emit(ctx, tc, latTB, xT8, w8d, bpkd, bseedd, outT, outR, h):
    import concourse.mybir as mybir

    nc = tc.nc
    F32 = mybir.dt.float32
    BF16 = mybir.dt.bfloat16
    FP8 = mybir.dt.float8e4
    AF = mybir.ActivationFunctionType
    ALU = mybir.AluOpType
    DR = mybir.MatmulPerfMode.DoubleRow

    const = ctx.enter_context(tc.tile_pool(name="const", bufs=1))
    # preload the tanh act table while the first DMAs are in flight
    with tc.high_priority():
        scr = const.tile([P, 2], F32, tag="scr")
        nc.vector.memset(scr[:, 0:1], 0.0)
        nc.scalar.activation(scr[:, 1:2], scr[:, 0:1],
                             mybir.ActivationFunctionType.Tanh)
    # fp8 weights (x8-scaled), stationary layout [K_lo, ktile, M]; w1 first so
    # the chain-bootstrap mini group can start before w2/w3 arrive
    w8sb = const.tile([P, 3, 2, D], FP8, tag="w8")
    nc.sync.dma_start(w8sb[:, 0, :, :],
                      w8d[0].rearrange("(k p) m -> p k m", k=2))
    # PSUM bias seeds for the chain: 8*b1 / 8*b2 / 8*b3, bcast along batch
    bseed = const.tile([P, 3, 2, P], BF16, tag="bseed")
    nc.scalar.dma_start(bseed[:], bseedd[:])
    for wi in (1, 2):
        nc.sync.dma_start(w8sb[:, wi, :, :],
                          w8d[wi].rearrange("(k p) m -> p k m", k=2))
    bsb = const.tile([P, 4], F32, tag="bias")
    nc.scalar.dma_start(bsb[:], bpkd[:])

    b1ap = [bsb[:, 0:1], bsb[:, 1:2]]
    b2ap = [bsb[:, 2:3], bsb[:, 3:4]]

    latp = ctx.enter_context(tc.tile_pool(name="lat", bufs=5))
    x8p = ctx.enter_context(tc.tile_pool(name="x8", bufs=5))
    h1p = ctx.enter_context(tc.tile_pool(name="h1", bufs=4))
    h2p = ctx.enter_context(tc.tile_pool(name="h2", bufs=4))
    oTp = ctx.enter_context(tc.tile_pool(name="oT", bufs=4))
    ringp = ctx.enter_context(tc.tile_pool(name="ring", bufs=1))
    y8pool = ctx.enter_context(tc.tile_pool(name="y8", bufs=4))
    c1sp = ctx.enter_context(tc.tile_pool(name="c1s", bufs=4))
    c2sp = ctx.enter_context(tc.tile_pool(name="c2s", bufs=4))
    minip = ctx.enter_context(tc.tile_pool(name="mini", bufs=1))

    # one PSUM bank per tile: fine-grained recycling so a stream matmul never
    # parks long at the PE queue head waiting for an act to free a bank
    mmps = ctx.enter_context(tc.tile_pool(name="mmps", bufs=6, space="PSUM"))
    chps = ctx.enter_context(tc.tile_pool(name="chps", bufs=2, space="PSUM"))

    h8 = float(h / 8.0)

    # ---- stream group stages -------------------------------------------
    def g_load(g):
        t0 = g * G
        xt = latp.tile([P, 2, G, P], F32, tag="lat")
        nc.sync.dma_start(xt[:], latTB[:, :, t0:t0 + G, :])
        x8 = x8p.tile([P, 2, G, P], FP8, tag="x8")
        nc.gpsimd.dma_start(x8[:], xT8[:, :, t0:t0 + G, :])
        return dict(xt=xt, x8=x8)

    def g_mm(s, wi, key, src):
        # N=256 halves: a chain matmul arriving at the PE queue waits out at
        # most ~256 rows of in-flight stream work instead of ~512
        mm = []
        NH = G * P // 2
        for mc in range(2):
            t = mmps.tile([P, G * P], F32, tag="mm", name=f"mm{wi}_{mc}")
            for nh in range(2):
                nc.tensor.matmul(t[:, nh * NH:(nh + 1) * NH],
                                 w8sb[:, wi, :, mc * P:(mc + 1) * P],
                                 src[:, :, nh * NH:(nh + 1) * NH],
                                 start=True, stop=True, perf_mode=DR)
            mm.append(t)
        s[key] = mm

    def g_act(s, mmkey, key, pool, bap):
        ht = pool.tile([P, 2, G * P], FP8, tag=key)
        for mc in range(2):
            nc.scalar.activation(ht[:, mc, :], s[mmkey][mc][:], AF.Tanh,
                                 bias=bap[mc], scale=0.125)
        s[key] = ht

    def g_store(s, g):
        t0 = g * G
        oT = oTp.tile([P, 2, G * P], F32, tag="oT")
        xtf = s["xt"].rearrange("p a t b -> p a (t b)")
        NH = G * P // 2
        for dc in range(2):
            for nh in range(2):
                sl = slice(nh * NH, (nh + 1) * NH)
                nc.vector.scalar_tensor_tensor(
                    oT[:, dc, sl], s["mm3"][dc][:, sl], h8, xtf[:, dc, sl],
                    ALU.mult, ALU.add)
        nc.sync.dma_start(outT[:, :, t0:t0 + G, :],
                          oT.rearrange("p a (t b) -> p a t b", t=G))
        s["oT"] = oT

    def g_all(s, g):
        g_mm(s, 0, "mm1", s["x8"].rearrange("p k t b -> p k (t b)"))
        g_act(s, "mm1", "h1", h1p, b1ap)
        g_mm(s, 1, "mm2", s["h1"])
        g_act(s, "mm2", "h2", h2p, b2ap)
        g_mm(s, 2, "mm3", s["h2"])
        g_store(s, g)

    # ---- chain stages ---------------------------------------------------
    def c_mm(wi, src, name):
        t = chps.tile([P, 2, P], F32, tag="ch", name=name)
        nc.vector.tensor_copy(t[:], bseed[:, wi, :, :])
        for mc in range(2):
            nc.tensor.matmul(t[:, mc, :],
                             w8sb[:, wi, :, mc * P:(mc + 1) * P],
                             src[:], start=False, stop=True, perf_mode=DR,
                             skip_group_check=True)
        return t

    def c_act(cin, pool, tag):
        t = pool.tile([P, 2, P], FP8, tag=tag)
        nc.scalar.activation(t[:], cin[:], AF.Tanh, scale=0.125)
        return t

    # ---- schedule -------------------------------------------------------
    # mini pre-group: compute y0 = out[:,100] from t=99 alone so the chain
    # starts ~6us in, instead of waiting for all of group 24
    with tc.high_priority():
        xt99 = minip.tile([P, 2, 1, P], F32, tag="xt99")
        nc.sync.dma_start(xt99[:], latTB[:, :, T_OBS - 1:T_OBS, :])
        x899 = minip.tile([P, 2, 1, P], FP8, tag="x899")
        nc.gpsimd.dma_start(x899[:], xT8[:, :, T_OBS - 1:T_OBS, :])
        m1 = c_mm(0, x899.rearrange("p a t b -> p a (t b)"), "m1")
        m1s = c_act(m1, c1sp, "c1s")
        m2 = c_mm(1, m1s, "m2")
        m2s = c_act(m2, c2sp, "c2s")
        m3 = chps.tile([P, 2, P], F32, tag="ch", name="m3")
        for mc in range(2):
            nc.tensor.matmul(m3[:, mc, :], w8sb[:, 2, :, mc * P:(mc + 1) * P],
                             m2s[:], start=True, stop=True, perf_mode=DR)
        y0 = minip.tile([P, 2, P], F32, tag="y0")
        nc.vector.scalar_tensor_tensor(y0[:], m3[:], h8,
                                       xt99.rearrange("p a t b -> p a (t b)"),
                                       ALU.mult, ALU.add)

        ring = ringp.tile([P, NCH, 2, P], F32, tag="ring")
        ysrc = y0[:, :, :]
        y8 = y8pool.tile([P, 2, P], FP8, tag="y8")
        nc.vector.tensor_copy(y8[:], ysrc)

    order = [NG - 1] + list(range(NG - 1))
    counts = [1] * 13 + [2] * 6          # 25 groups over 19 chain slots
    assert sum(counts) == len(order)
    states = {}
    nloaded = 0

    def prefetch(upto):
        nonlocal nloaded
        while nloaded < min(upto, len(order)):
            states[order[nloaded]] = g_load(order[nloaded])
            nloaded += 1

    prefetch(1)
    done = 0
    for k in range(NCH):
        gs = [states[order[done + i]] for i in range(counts[k])]
        done += counts[k]
        prefetch(done + counts[min(k + 1, NCH - 1)] + 1)
        with tc.high_priority():
            c1 = c_mm(0, y8, "c1")
        for s in gs:
            g_mm(s, 0, "mm1", s["x8"].rearrange("p k t b -> p k (t b)"))
        with tc.high_priority():
            c1s = c_act(c1, c1sp, "c1s")
        for s in gs:
            g_act(s, "mm1", "h1", h1p, b1ap)
        with tc.high_priority():
            c2 = c_mm(1, c1s, "c2")
        for s in gs:
            g_mm(s, 1, "mm2", s["h1"])
        with tc.high_priority():
            c2s = c_act(c2, c2sp, "c2s")
        for s in gs:
            g_act(s, "mm2", "h2", h2p, b2ap)
        with tc.high_priority():
            c3 = c_mm(2, c2s, "c3")
        for s in gs:
            g_mm(s, 2, "mm3", s["h2"])
        ynew = ring[:, k, :, :]
        with tc.high_priority():
            if k < NCH - 1:
                y8n = y8pool.tile([P, 2, P], FP8, tag="y8")
                nc.vector.scalar_tensor_tensor(y8n[:], c3[:], h8, ysrc,
                                               ALU.mult, ALU.add)
                y8 = y8n
            nc.vector.scalar_tensor_tensor(ynew, c3[:], h8, ysrc,
                                           ALU.mult, ALU.add)
        for i, s in enumerate(gs):
            g_store(s, order[done - counts[k] + i])
        ysrc = ynew
        if k == 12:
            nc.gpsimd.dma_start(outR[:, 0:12, :, :], ring[:, 0:12, :, :])
        elif k == 16:
            nc.gpsimd.dma_start(outR[:, 12:16, :, :], ring[:, 12:16, :, :])

    nc.gpsimd.dma_start(outR[:, 16:, :, :], ring[:, 16:, :, :])


def _build(h):
    import concourse.mybir as mybir
    import concourse.tile as tile
    from concourse import bacc

    F32 = mybir.dt.float32
    BF16 = mybir.dt.bfloat16
    FP8 = mybir.dt.float8e4

    nc = bacc.Bacc("TRN2", target_bir_lowering=False, debug=False,
                   num_devices=NCORES)
    latTB = nc.dram_tensor("latTB", [P, 2, T_OBS, PB], F32,
                           kind="ExternalInput").ap()
    xT8 = nc.dram_tensor("xT8", [P, 2, T_OBS, PB], FP8,
                         kind="ExternalInput").ap()
    w8d = nc.dram_tensor("w8", [3, D, D], FP8, kind="ExternalInput").ap()
    bpkd = nc.dram_tensor("bpack", [P, 4], F32, kind="ExternalInput").ap()
    bseedd = nc.dram_tensor("bseed", [P, 3, 2, PB], BF16,
                            kind="ExternalInput").ap()
    outT = nc.dram_tensor("outT", [P, 2, T_OBS, PB], F32,
                          kind="ExternalOutput").ap()
    outR = nc.dram_tensor("outR", [P, NCH, 2, PB], F32,
                          kind="ExternalOutput").ap()

    with tile.TileContext(nc) as tc, ExitStack() as ctx:
        _emit(ctx, tc, latTB, xT8, w8d, bpkd, bseedd, outT, outR, h)
    nc.compile()
    return nc


def _host_inputs(inputs):
    ts = np.asarray(inputs["time_steps"], np.float32)
    h = float(np.float32(ts[1]) - np.float32(ts[0]))

    f8 = ml_dtypes.float8_e4m3
    W1 = np.asarray(inputs["W1"], np.float32)
    W2 = np.asarray(inputs["W2"], np.float32)
    W3 = np.asarray(inputs["W3"], np.float32)
    b1 = np.asarray(inputs["b1"], np.float32)
    b2 = np.asarray(inputs["b2"], np.float32)
    b3 = np.asarray(inputs["b3"], np.float32)
    w8 = np.stack([8.0 * W1, 8.0 * W2, 8.0 * W3]).astype(f8)
    b3h = (b3 * np.float32(h)).astype(np.float32)
    bpack = np.stack([b1[:P], b1[P:], b2[:P], b2[P:]],
                     axis=1).astype(np.float32)
    # [P, 3, 2, PB]: 8*b{1,2,3}[mc*128+p] broadcast along batch
    bs = np.stack([8.0 * b1, 8.0 * b2, 8.0 * b3])        # [3, 256]
    bseed = np.ascontiguousarray(
        np.broadcast_to(bs.reshape(3, 2, P, 1).transpose(2, 0, 1, 3),
                        (P, 3, 2, PB))).astype(ml_dtypes.bfloat16)
    shared = dict(w8=w8, bpack=bpack, bseed=bseed)
    return h, shared, b3h


def _percore_inputs(lat_full, b3h):
    # lat_full [B, T_OBS, D] -> per-core latTB/xT8 [P, 2, T_OBS, PB]
    f8 = ml_dtypes.float8_e4m3
    x = lat_full.reshape(NCORES, PB, T_OBS, 2, P)   # [c, b, t, dc, p]
    xt = x.transpose(0, 4, 3, 2, 1)                 # [c, p, dc, t, b]
    b3r = b3h.reshape(2, P).transpose(1, 0)         # [p, dc]
    latTBs = (xt + b3r[None, :, :, None, None]).astype(np.float32)
    xT8s = np.ascontiguousarray(xt).astype(f8)
    return latTBs, xT8s


def _assemble(lat_full, results):
    out = np.empty((B, T, D), np.float32)
    for c in range(NCORES):
        sl = slice(c * PB, (c + 1) * PB)
        oT = results[c]["outT"]    # [P, 2, T_OBS, PB]
        oR = results[c]["outR"]    # [P, NCH, 2, PB]
        out[sl, 1:T_OBS + 1, :] = oT.transpose(3, 2, 1, 0).reshape(
            PB, T_OBS, D)
        out[sl, T_OBS + 1:, :] = oR.transpose(3, 1, 2, 0).reshape(PB, NCH, D)
    out[:, 0, :] = lat_full[:, 0, :]
    out[:, 2, :] = lat_full[:, 1, :]
    return out


_CACHE = {}


def make_in_maps(inputs):
    lat_full = np.ascontiguousarray(np.asarray(inputs["latents"], np.float32))
    h, shared, b3h = _host_inputs(inputs)
    if h not in _CACHE:
        _CACHE[h] = _build(h)
    nc = _CACHE[h]
    latTBs, xT8s = _percore_inputs(lat_full, b3h)
    in_maps = []
    for c in range(NCORES):
        m = dict(shared)
        m["latTB"] = np.ascontiguousarray(latTBs[c])
        m["xT8"] = xT8s[c]
        in_maps.append(m)
    return nc, in_maps, lat_full


def kernel(**inputs):
    from concourse.bass_utils import run_bass_kernel_spmd

    nc, in_maps, lat_full = make_in_maps(inputs)
    res = run_bass_kernel_spmd(nc, in_maps, list(range(NCORES)))
    return _assemble(lat_full, [res.results[c] for c in range(NCORES)])
